# revision 12
# baseline (speedup 1.0000x reference)
"""Trainium2 Bass kernel for nn_LogActivationLayer.

y[b,o] = sum_i gamma[o,i]/64 * ( b1*log(1 + b2*log(1 + (exp(b3*x[b,i])-1)**b4))
                                 + b5*x + b6*x^2 + b7*x^3 + b8*x^4 )
with x = relu(x), and b1..b8 = spline tables evaluated at w_norm[o,i]
(host-precomputable: they depend only on the tiny [64,64] parameters).

Sharding: each of the 8 cores owns 8 of the 64 output channels (the sum
over `in` is core-local; x is replicated).  Per core the [out,in] pairs
form 4 partition-tiles of 128 (= 2 outs x 64 ins) x 8192 batch columns.

Per tile the log term is exactly 5 ACT passes, all in the
natural_log_exp_and_others table set (no table switches):
    e  = Exp(b3 * x)            (per-partition scale operand)
    l  = Ln(max(e - 1, 1e-30))  (DVE clamp between)
    p  = Exp(b4 * l)
    L1 = Ln(p + 1)
    L  = Ln(b2*L1 + 1)
The multiply by b1*gamma/64 and the sum over `in` fold into a
block-structured PE matmul; the polynomial terms are 4 more matmuls
(x, x^2, x^3, x^4 against folded weights), all accumulating in PSUM.
"""

import sys

import ml_dtypes
import numpy as np

for _p in ("/opt/trn_rl_repo",):
    if _p not in sys.path:
        sys.path.append(_p)

import concourse.bass as bass
import concourse.tile as tile
from concourse import mybir
from concourse.bass_utils import run_bass_kernel_spmd

B, IN, OUT = 8192, 64, 64
N_CORES = 8
O_PER = OUT // N_CORES      # 8 output channels per core
NT = O_PER // 2             # 4 pair-tiles (2 outs x 64 ins = 128 partitions)
CHUNKS = [1024, 3072, 4096]  # uneven batch chunks: small first => ACT starts early
PSN = 2048                  # psum accumulation chunk (4 banks)
MMN = 512                   # matmul max moving free dim
EPS = 1e-30

F32 = mybir.dt.float32
BF16 = mybir.dt.bfloat16


def _split_sync_waits(nc, max_waits=1):
    """This container's walrus rejects >1 sem-wait per instruction; hoist
    excess waits onto same-engine NoOps inserted just before."""
    n = 0
    for fn in nc.m.functions:
        for blk in fn.blocks:
            insts = getattr(blk, "instructions", None)
            if not insts:
                continue
            out = []
            for inst in insts:
                si = getattr(inst, "sync_info", None)
                if si is not None and si.on_wait and len(si.on_wait) > max_waits:
                    waits = list(si.on_wait)
                    extra, keep = waits[:-max_waits], waits[-max_waits:]
                    for w in extra:
                        n += 1
                        out.append(
                            mybir.InstNoOp(
                                name=f"{inst.name}-sw{n}",
                                engine=inst.engine,
                                bass_nofuse=True,
                                sync_info=mybir.SyncInfo(on_wait=[w], on_update=[]),
                            )
                        )
                    si.on_wait = keep
                out.append(inst)
            blk.instructions = out
    return n


def _build_nc():
    FT = mybir.ActivationFunctionType
    OP = mybir.AluOpType
    nc = bass.Bass("TRN2", target_bir_lowering=False)

    xt = nc.dram_tensor("xt", [IN, B], F32, kind="ExternalInput")
    b3v = nc.dram_tensor("b3v", [128, NT], F32, kind="ExternalInput")
    b4v = nc.dram_tensor("b4v", [128, NT], F32, kind="ExternalInput")
    b2v = nc.dram_tensor("b2v", [128, NT], F32, kind="ExternalInput")
    c1w = nc.dram_tensor("c1w", [128, NT * O_PER], BF16, kind="ExternalInput")
    pw = nc.dram_tensor("pw", [128, 2 * O_PER], BF16, kind="ExternalInput")
    yt = nc.dram_tensor("yt", [O_PER, B], F32, kind="ExternalOutput")

    with tile.TileContext(nc) as tc:
        with (
            tc.tile_pool(name="consts", bufs=1) as consts,
            tc.tile_pool(name="xp", bufs=2) as xp,
            tc.tile_pool(name="powp", bufs=1) as powp,
            tc.tile_pool(name="pxp", bufs=1) as pxp,
            tc.tile_pool(name="chain", bufs=3) as chain,
            tc.tile_pool(name="chb", bufs=5) as chb,
            tc.tile_pool(name="yc", bufs=2) as ycp,
            tc.tile_pool(name="ps", bufs=2, space="PSUM") as psp,
        ):
            # dummy activation at t=0: pulls the exp/ln table load off the
            # critical path (overlaps the input DMA)
            warm = consts.tile([128, 1], F32)
            nc.vector.memset(warm[:], 0.0)
            nc.scalar.activation(out=warm[:], in_=warm[:], func=FT.Exp, bias=0.0)

            b3s = consts.tile([128, NT], F32)
            nc.gpsimd.dma_start(out=b3s[:], in_=b3v[:])
            b4s = consts.tile([128, NT], F32)
            nc.gpsimd.dma_start(out=b4s[:], in_=b4v[:])
            b2s = consts.tile([128, NT], F32)
            nc.gpsimd.dma_start(out=b2s[:], in_=b2v[:])
            c1s = consts.tile([128, NT * O_PER], BF16)
            nc.gpsimd.dma_start(out=c1s[:], in_=c1w[:])
            pws = consts.tile([128, 2 * O_PER], BF16)
            nc.gpsimd.dma_start(out=pws[:], in_=pw[:])

            lo = 0
            for FCH in CHUNKS:
                xsb = xp.tile([128, FCH], F32)
                nc.sync.dma_start(out=xsb[0:IN, :], in_=xt[:, lo : lo + FCH])
                # duplicate to upper partitions + relu both halves
                nc.vector.tensor_scalar_max(
                    out=xsb[IN:128, :], in0=xsb[0:IN, :], scalar1=0.0
                )
                nc.vector.tensor_scalar_max(
                    out=xsb[0:IN, :], in0=xsb[0:IN, :], scalar1=0.0
                )
                xsq = powp.tile([IN, FCH], F32)
                nc.vector.tensor_mul(out=xsq[:], in0=xsb[0:IN, :], in1=xsb[0:IN, :])
                # stacked bf16 power tiles: px1 = [x; x^2], px2 = [x^3; x^4]
                px1 = pxp.tile([128, FCH], BF16, tag="px1")
                nc.vector.tensor_copy(out=px1[0:IN, :], in_=xsb[0:IN, :])
                nc.vector.tensor_mul(
                    out=px1[IN:128, :], in0=xsb[0:IN, :], in1=xsb[0:IN, :]
                )
                px2 = pxp.tile([128, FCH], BF16, tag="px2")
                nc.vector.tensor_mul(out=px2[0:IN, :], in0=xsq[:], in1=xsb[0:IN, :])
                nc.vector.tensor_mul(out=px2[IN:128, :], in0=xsq[:], in1=xsq[:])

                As = []
                for t in range(NT):
                    A = chain.tile([128, FCH], F32)
                    nc.scalar.activation(
                        out=A[:], in_=xsb[:], func=FT.Exp, bias=0.0,
                        scale=b3s[:, t : t + 1],
                    )
                    nc.vector.tensor_scalar(
                        out=A[:], in0=A[:], scalar1=-1.0, scalar2=EPS,
                        op0=OP.add, op1=OP.max,
                    )
                    nc.scalar.activation(out=A[:], in_=A[:], func=FT.Ln, bias=0.0)
                    nc.scalar.activation(
                        out=A[:], in_=A[:], func=FT.Exp, bias=0.0,
                        scale=b4s[:, t : t + 1],
                    )
                    nc.scalar.activation(out=A[:], in_=A[:], func=FT.Ln, bias=1.0)
                    Ab = chb.tile([128, FCH], BF16)
                    nc.scalar.activation(
                        out=Ab[:], in_=A[:], func=FT.Ln, bias=1.0,
                        scale=b2s[:, t : t + 1],
                    )
                    As.append(Ab)

                for h, hc in enumerate(range(0, FCH, PSN)):
                    pn = min(PSN, FCH - hc)
                    ps = psp.tile([O_PER, pn], F32, tag="ps")
                    for n in range(pn // MMN):
                        col = hc + n * MMN
                        pc = n * MMN
                        mms = [
                            (pws[:, 0:O_PER], px1[:, col : col + MMN]),
                            (pws[:, O_PER : 2 * O_PER], px2[:, col : col + MMN]),
                        ] + [
                            (c1s[:, t * O_PER : (t + 1) * O_PER],
                             As[t][:, col : col + MMN])
                            for t in range(NT)
                        ]
                        for k, (lhsT, rhs) in enumerate(mms):
                            nc.tensor.matmul(
                                ps[:, pc : pc + MMN], lhsT, rhs,
                                start=(k == 0), stop=(k == len(mms) - 1),
                            )
                    yc = ycp.tile([O_PER, pn], F32, tag="yc")
                    nc.vector.tensor_copy(out=yc[:], in_=ps[:])
                    nc.sync.dma_start(
                        out=yt[:, lo + hc : lo + hc + pn], in_=yc[:]
                    )
                lo += FCH

    _split_sync_waits(nc)
    return nc


_NC_CACHE = {}


def _get_nc():
    if "nc" not in _NC_CACHE:
        _NC_CACHE["nc"] = _build_nc()
    return _NC_CACHE["nc"]


def _eval_splines(w, breaks, coefs, mu, sigma):
    """b[s,o,i] = spline_s(w_norm[o,i]); mirrors reference in float32."""
    w_c = np.clip(w, -5.5, 37.9).astype(np.float32)
    w_norm = ((w_c - np.float32(mu)) / np.float32(sigma)).astype(np.float32)
    bs = []
    for s in range(breaks.shape[0]):
        br = breaks[s]
        cf = coefs[s]
        wl = np.clip(w_norm, br[0], br[-1] - np.float32(1e-6))
        idx = np.clip(np.searchsorted(br, wl, side="left") - 1, 0, cf.shape[0] - 1)
        a = cf[idx]
        t = (wl - br[idx]).astype(np.float32)
        bs.append(((a[..., 0] * t + a[..., 1]) * t + a[..., 2]) * t + a[..., 3])
    return np.stack(bs).astype(np.float32)


def _host_params(raw_gamma, w, breaks, coefs, mu, sigma):
    b = _eval_splines(w, breaks, coefs, mu, sigma)  # [8, OUT, IN]
    b1, b2, b3, b4, b5, b6, b7, b8 = b
    gamma = np.log1p(np.exp(raw_gamma.astype(np.float32))).astype(np.float32)
    scale = (gamma / np.float32(OUT)).astype(np.float32)
    c1 = (b1 * scale).astype(np.float32)
    cps = [(bp * scale).astype(np.float32) for bp in (b5, b6, b7, b8)]
    return b2, b3, b4, c1, cps


def _core_inputs(xtc, b2, b3, b4, c1, cps, c):
    o0 = c * O_PER

    def pairs(m):  # [OUT,IN] -> [128, NT] per-partition vectors for this core
        return np.ascontiguousarray(m[o0 : o0 + O_PER].reshape(NT, 128).T)

    c1w = np.zeros((128, NT * O_PER), dtype=np.float32)
    for t in range(NT):
        c1w[0:IN, t * O_PER + 2 * t] = c1[o0 + 2 * t]
        c1w[IN:128, t * O_PER + 2 * t + 1] = c1[o0 + 2 * t + 1]
    # pw: lhsT for stacked power tiles px1=[x;x^2], px2=[x^3;x^4]
    pwm = np.zeros((128, 2 * O_PER), dtype=np.float32)
    pwm[0:IN, 0:O_PER] = cps[0][o0 : o0 + O_PER].T        # c5 vs x
    pwm[IN:128, 0:O_PER] = cps[1][o0 : o0 + O_PER].T      # c6 vs x^2
    pwm[0:IN, O_PER : 2 * O_PER] = cps[2][o0 : o0 + O_PER].T    # c7 vs x^3
    pwm[IN:128, O_PER : 2 * O_PER] = cps[3][o0 : o0 + O_PER].T  # c8 vs x^4
    return {
        "xt": xtc,
        "b3v": pairs(b3),
        "b4v": pairs(b4),
        "b2v": pairs(b2),
        "c1w": c1w.astype(ml_dtypes.bfloat16),
        "pw": pwm.astype(ml_dtypes.bfloat16),
    }


def kernel(x, raw_gamma, w, breaks, coefs, mu_detuning, sigma_detuning):
    b2, b3, b4, c1, cps = _host_params(
        raw_gamma, w, breaks, coefs, mu_detuning, sigma_detuning
    )
    xtc = np.ascontiguousarray(x.astype(np.float32).T)  # [IN, B]
    in_maps = [_core_inputs(xtc, b2, b3, b4, c1, cps, c) for c in range(N_CORES)]
    nc = _get_nc()
    res = run_bass_kernel_spmd(nc, in_maps, core_ids=list(range(N_CORES)))
    y = np.empty((B, OUT), dtype=np.float32)
    for c in range(N_CORES):
        y[:, c * O_PER : (c + 1) * O_PER] = res.results[c]["yt"].T
    return y
